# revision 7
# baseline (speedup 1.0000x reference)
"""Confusion-matrix (150x150) histogram kernel for Trainium2, 8 NeuronCores.

Algorithm
---------
cm[t, p] += 1 for 8.4M (t, p) pairs == histogram over 22500 bins of
bin = t*150 + p.  Data-parallel over 8 cores (1M elements each).

On-device per core: no scatter exists on TRN2, so counting is done as a
one-hot outer-product accumulated by the tensor engine:

    bin = t*150 + p          (DVE, exact: products <= 22350)
    v   = bin & 127          (128-wide one-hot -> matmul lhsT)
    u   = bin >> 7           (176-wide one-hot -> matmul rhs)
    psum[v, u] += onehot(v)^T @ onehot(u)   (PSUM f32, exact integer adds)

One-hots are built per 128-element chunk with DVE tensor_scalar(is_equal)
against a bf16 iota (bf16 => 4x DVE mode).  Host unpacks psum[v, u] into
counts[bin = u*128 + v] and sums partials from the 8 cores.
"""

import numpy as np

NUM_CLASSES = 150
N = 8_388_608
N_CORES = 8
P = 128
PER_CORE = N // N_CORES          # 1_048_576
E = PER_CORE // P                # 8192 elements per partition row
TILE_E = 512                     # elements-per-partition per DMA/prep tile
NT = E // TILE_E                 # 16 tiles
VW = 128                         # v = bin & 127  (one-hot width, lhsT/out partitions)
UW = 176                         # u = bin >> 7   (one-hot width, rhs/out free dim; 22500>>7 = 175)
NCHUNK = E                       # one chunk = one column of 128 elements


_cached_nc = None


def _build_module():
    global _cached_nc
    if _cached_nc is not None:
        return _cached_nc
    from contextlib import ExitStack

    import concourse.bass as bass
    import concourse.tile as tile
    from concourse import bacc, mybir

    nc = bacc.Bacc(
        "TRN2",
        target_bir_lowering=False,
        debug=False,
        enable_asserts=False,
        num_devices=N_CORES,
    )
    t_d = nc.dram_tensor("t", [P, E], mybir.dt.int32, kind="ExternalInput")
    p_d = nc.dram_tensor("p", [P, E], mybir.dt.int32, kind="ExternalInput")
    out_d = nc.dram_tensor("out", [P, UW], mybir.dt.float32, kind="ExternalOutput")

    i32 = mybir.dt.int32
    bf16 = mybir.dt.bfloat16
    f32 = mybir.dt.float32
    Op = mybir.AluOpType

    with tile.TileContext(nc) as tc, ExitStack() as ctx:
        const_pool = ctx.enter_context(tc.tile_pool(name="const", bufs=1))
        io_pool = ctx.enter_context(tc.tile_pool(name="io", bufs=3))
        prep_pool = ctx.enter_context(tc.tile_pool(name="prep", bufs=2))
        oh_pool = ctx.enter_context(tc.tile_pool(name="oh", bufs=8))
        psum_pool = ctx.enter_context(tc.tile_pool(name="psum", bufs=1, space="PSUM"))

        iota_i = const_pool.tile([P, UW], i32)
        nc.gpsimd.iota(iota_i[:], pattern=[[1, UW]], base=0, channel_multiplier=0)
        iota_bf = const_pool.tile([P, UW], bf16)
        nc.vector.tensor_copy(iota_bf[:], iota_i[:])

        psum = psum_pool.tile([P, UW], f32)

        chunk = 0
        for it in range(NT):
            t_t = io_pool.tile([P, TILE_E], i32, tag="tin")
            nc.sync.dma_start(t_t[:], t_d.ap()[:, bass.ts(it, TILE_E)])
            p_t = io_pool.tile([P, TILE_E], i32, tag="pin")
            nc.sync.dma_start(p_t[:], p_d.ap()[:, bass.ts(it, TILE_E)])

            # bin = t*150 + p   (int32; exact)
            bin_t = prep_pool.tile([P, TILE_E], i32, tag="bin")
            nc.vector.scalar_tensor_tensor(
                bin_t[:], t_t[:], 150, p_t[:], op0=Op.mult, op1=Op.add
            )
            u_i = prep_pool.tile([P, TILE_E], i32, tag="ui")
            nc.vector.tensor_scalar(
                u_i[:], bin_t[:], 7, None, op0=Op.logical_shift_right
            )
            v_i = prep_pool.tile([P, TILE_E], i32, tag="vi")
            nc.vector.tensor_scalar(v_i[:], bin_t[:], 127, None, op0=Op.bitwise_and)
            u_sc = prep_pool.tile([P, TILE_E], f32, tag="ub")
            nc.vector.tensor_copy(u_sc[:], u_i[:])
            v_sc = prep_pool.tile([P, TILE_E], f32, tag="vb")
            nc.vector.tensor_copy(v_sc[:], v_i[:])

            for e in range(TILE_E):
                oh_v = oh_pool.tile([P, VW], bf16, tag="ohv")
                nc.vector.tensor_scalar(
                    oh_v[:], iota_bf[:, 0:VW], v_sc[:, e : e + 1], None,
                    op0=Op.is_equal,
                )
                oh_u = oh_pool.tile([P, UW], bf16, tag="ohu")
                nc.vector.tensor_scalar(
                    oh_u[:], iota_bf[:], u_sc[:, e : e + 1], None,
                    op0=Op.is_equal,
                )
                nc.tensor.matmul(
                    psum[:],
                    oh_v[:],
                    oh_u[:],
                    start=(chunk == 0),
                    stop=(chunk == NCHUNK - 1),
                )
                chunk += 1

        out_sb = const_pool.tile([P, UW], f32)
        nc.vector.tensor_copy(out_sb[:], psum[:])
        nc.sync.dma_start(out_d.ap()[:, :], out_sb[:])

    nc.compile()
    _cached_nc = nc
    return nc


def _ensure_axon_hooks_stub():
    # Under axon, run_bass_kernel_spmd(trace=True) imports antenv.axon_hooks;
    # some containers ship only the antenv stub. Provide a no-hook fallback so
    # a BASS_TRACE=1 environment degrades to untraced instead of crashing.
    try:
        import antenv.axon_hooks  # noqa: F401
    except ImportError:
        import sys
        import types

        mod = types.ModuleType("antenv.axon_hooks")
        mod.get_axon_ntff_profile_hook = lambda: None
        sys.modules["antenv.axon_hooks"] = mod


def kernel(confusion_matrix, predictions, targets):
    from concourse import bass_utils

    _ensure_axon_hooks_stub()

    preds = np.ascontiguousarray(np.asarray(predictions).astype(np.int32))
    targs = np.ascontiguousarray(np.asarray(targets).astype(np.int32))
    cm_in = np.asarray(confusion_matrix, dtype=np.float32)
    assert preds.shape == (N,) and targs.shape == (N,)

    nc = _build_module()

    in_maps = []
    for c in range(N_CORES):
        sl = slice(c * PER_CORE, (c + 1) * PER_CORE)
        in_maps.append(
            {
                "t": targs[sl].reshape(P, E),
                "p": preds[sl].reshape(P, E),
            }
        )

    res = bass_utils.run_bass_kernel_spmd(
        nc, in_maps, core_ids=list(range(N_CORES))
    )
    global _last_results, _last_nc
    _last_results = res
    _last_nc = nc

    counts = np.zeros((NUM_CLASSES * NUM_CLASSES,), dtype=np.float64)
    for c in range(N_CORES):
        part = res.results[c]["out"]          # [VW=128, UW=176], part[v, u]
        flat = part.T.reshape(-1)             # index u*128 + v == bin
        counts += flat[: NUM_CLASSES * NUM_CLASSES].astype(np.float64)

    out = cm_in + counts.reshape(NUM_CLASSES, NUM_CLASSES).astype(np.float32)
    return out.astype(np.float32)
